# revision 35
# baseline (speedup 1.0000x reference)
"""Trainium2 Bass kernel: 16-head MHA (B=2, S=2048, D=1024) on 8 NeuronCores.

Sharding: core c handles batch c//4 and heads 4*(c%4) .. 4*(c%4)+3
(data parallel over batch, tensor parallel over heads). Q/K/V projections
are column-sharded by head, the output projection is row-sharded; each
core emits a partial (S, D) output (fp16) and the host sums the 4
partials per batch.

Per-core pipeline (all matmul inputs bf16, fp32 PSUM accumulation):
  - host supplies x^T (c-major) per input and pre-transposed weight slices
  - q/k projections produce qT/kT [128, S] per head-pair (head dim on
    partitions: pair p holds head 2p at partitions 0-63, head 2p+1 at
    64-127)
  - v projection produces V natural [S, 256] directly (s on partitions),
    augmented with a ones column per head for softmax denominators
  - attention runs one unified 128-item stream over (pair, qc, kb):
    the two heads of a pair issue S^T matmuls with K=64 at row
    tile_positions (0,0)/(64,0) back-to-back so they execute
    CONCURRENTLY on the PE array; both land in one 2-bank PSUM group
    and a single 1024-wide exp on ACT converts the group (halving
    ACT instruction overhead vs per-tile exps)
  - PV per head: P^T@V' with the ones column gives O^T[d,q] plus the
    denominator row; PVs lag the exp stream and are emitted BEFORE the
    S matmuls of each item so a stalled S never head-of-line-blocks them
  - q1/k1 projections, the v projection, and the output projection are
    woven into the stream as weave units to fill PE idle under the
    ACT-bound exp stream
  - output projection consumes atm per qc block as soon as both pairs'
    norms for that qc are done; y is fp16 to halve the output DMA
"""

import sys

import numpy as np
import ml_dtypes

if "/opt/trn_rl_repo" not in sys.path:
    sys.path.insert(0, "/opt/trn_rl_repo")

B, S, D = 2, 2048, 1024
H, DK = 16, 64
NCORES = 8
HL = 4            # heads per core
DL = HL * DK      # 256 local projection dims
SCALE = 1.0 / 8.0  # 1/sqrt(DK)
LAG = 12          # PV items lag the S/exp stream

_CACHE = {}


def _build_nc():
    import concourse.bass as bass  # noqa: F401
    import concourse.mybir as mybir
    from concourse import bacc, tile

    f32 = mybir.dt.float32
    f16 = mybir.dt.float16
    bf16 = mybir.dt.bfloat16
    AF = mybir.ActivationFunctionType

    nc = bacc.Bacc(None, target_bir_lowering=False, debug=False)
    xqT = nc.declare_dram_parameter("xqT", [D, S], bf16, isOutput=False)
    xkT = nc.declare_dram_parameter("xkT", [D, S], bf16, isOutput=False)
    xvT = nc.declare_dram_parameter("xvT", [D, S], bf16, isOutput=False)
    wqT = nc.declare_dram_parameter("wqT", [D, DL], bf16, isOutput=False)
    wkT = nc.declare_dram_parameter("wkT", [D, DL], bf16, isOutput=False)
    wvT = nc.declare_dram_parameter("wvT", [D, DL], bf16, isOutput=False)
    woT = nc.declare_dram_parameter("woT", [DL, D], bf16, isOutput=False)
    y = nc.declare_dram_parameter("y", [S, D], f16, isOutput=True)

    with tile.TileContext(nc) as tc, \
         tc.tile_pool(name="singles", bufs=1) as singles, \
         tc.tile_pool(name="psum", bufs=1, space="PSUM") as pp, \
         tc.tile_pool(name="work", bufs=1) as wk, \
         tc.tile_pool(name="dram", bufs=1, space="DRAM") as adr:
        # PSUM: 8 banks = sg 2x2 (S-pair groups, double-buffered)
        #               + ot 2 (PV accumulators for the live pair)
        #               + pj 2 (projection / output-projection)
        SG = dict(tag="sg", bufs=2)    # [128, 2, 512] f32 = 2 banks each
        OT = dict(tag="ot", bufs=2)    # [128, 512] f32 = 1 bank each
        PJ = dict(tag="pj", bufs=2)    # [128, 512] f32 = 1 bank each

        wq_sb = singles.tile([128, 8, DL], bf16)
        wk_sb = singles.tile([128, 8, DL], bf16)
        wv_sb = singles.tile([128, 8, DL], bf16)
        wo_sb = singles.tile([128, 2, D], bf16)
        qTm = [singles.tile([128, S], bf16, name=f"qT{m}") for m in range(2)]
        kTm = [singles.tile([128, S], bf16, name=f"kT{m}") for m in range(2)]
        atm = [singles.tile([128, S], bf16, name=f"at{m}") for m in range(2)]
        # V' per k-block: [128, pair, 2 heads x (64 v cols + ones)]
        vpst = [singles.tile([128, 2, 130], bf16, name=f"vp{st}")
                for st in range(16)]
        for st in range(16):
            nc.vector.memset(
                vpst[st].rearrange("p m (h e) -> p m h e", e=65)[:, :, :, 64:65],
                1.0)

        xq_sl = [singles.tile([128, S], bf16, name=f"xq{i}") for i in range(8)]
        xk_sl = [singles.tile([128, S], bf16, name=f"xk{i}") for i in range(8)]
        xv_sl = [singles.tile([128, S], bf16, name=f"xv{i}") for i in range(8)]
        # DMA order = need order: wq+xq (q0 proj), wk+xk (k0), wv+xv, wo
        nc.sync.dma_start(wq_sb, wqT.rearrange("(ct p) e -> p ct e", p=128))
        for ct in range(8):
            nc.sync.dma_start(xq_sl[ct], xqT[ct * 128:(ct + 1) * 128, :])
        nc.sync.dma_start(wk_sb, wkT.rearrange("(ct p) e -> p ct e", p=128))
        for ct in range(8):
            nc.sync.dma_start(xk_sl[ct], xkT[ct * 128:(ct + 1) * 128, :])
        nc.sync.dma_start(wv_sb, wvT.rearrange("(ct p) e -> p ct e", p=128))
        for ct in range(8):
            nc.sync.dma_start(xv_sl[ct], xvT[ct * 128:(ct + 1) * 128, :])
        nc.sync.dma_start(wo_sb, woT.rearrange("(ct p) e -> p ct e", p=128))

        # ---------------- startup projections ----------------
        # q0 AND q1 per-ct during the xq DMA window (q0 -> the sg group
        # tiles, q1 -> ot+pj banks), then k0 during the xk window
        # (reusing freshly-freed sg-tag tiles); per-ct emission so
        # compute tracks DMA slab arrival.
        sg_q = [pp.tile([128, 2, 512], f32, name=f"sgq{i}", **SG)
                for i in range(2)]
        ps_q0 = [sg_q[i][:, hh, :] for i in range(2) for hh in range(2)]
        ps_q1 = [pp.tile([128, 512], f32, name=f"psq1_{i}",
                         **(OT if i < 2 else PJ))[:] for i in range(4)]
        for ct in range(8):
            for n in range(4):
                nc.tensor.matmul(
                    ps_q0[n],
                    lhsT=wq_sb[:, ct, 0:128],
                    rhs=xq_sl[ct][:, n * 512:(n + 1) * 512],
                    start=(ct == 0), stop=(ct == 7),
                )
            for n in range(4):
                nc.tensor.matmul(
                    ps_q1[n],
                    lhsT=wq_sb[:, ct, 128:256],
                    rhs=xq_sl[ct][:, n * 512:(n + 1) * 512],
                    start=(ct == 0), stop=(ct == 7),
                )
        for n in range(4):
            nc.vector.tensor_copy(qTm[0][:, n * 512:(n + 1) * 512], ps_q0[n])
            nc.vector.tensor_copy(qTm[1][:, n * 512:(n + 1) * 512], ps_q1[n])
        sg_k = [pp.tile([128, 2, 512], f32, name=f"sgk{i}", **SG)
                for i in range(2)]
        ps_k0 = [sg_k[i][:, hh, :] for i in range(2) for hh in range(2)]
        for ct in range(8):
            for n in range(4):
                nc.tensor.matmul(
                    ps_k0[n],
                    lhsT=wk_sb[:, ct, 0:128],
                    rhs=xk_sl[ct][:, n * 512:(n + 1) * 512],
                    start=(ct == 0), stop=(ct == 7),
                )
        for n in range(4):
            nc.vector.tensor_copy(kTm[0][:, n * 512:(n + 1) * 512], ps_k0[n])

        # ---------------- weave units ----------------
        def v_unit(st):
            vt = pp.tile([128, 512], f32, name=f"vP{st}", **PJ)
            for ct in range(8):
                nc.tensor.matmul(
                    vt[:, 0:256],
                    lhsT=xv_sl[ct][:, st * 128:(st + 1) * 128],
                    rhs=wv_sb[:, ct, :],
                    start=(ct == 0), stop=(ct == 7),
                )
            for m in range(2):
                nc.vector.tensor_copy(
                    vpst[st].rearrange("p m (h e) -> p m h e",
                                       e=65)[:, m, :, 0:64],
                    vt[:, m * 128:(m + 1) * 128].rearrange(
                        "p (h d) -> p h d", d=64),
                )

        # k1 projection (q1 ran at startup): 9 units per half: 8 ct units
        # (2 matmuls) + 1 copy unit
        pj_ps = {}

        def qk1_unit(u):
            half, step = divmod(u, 9)
            key = half
            if step == 0:
                pj_ps[key] = [pp.tile([128, 512], f32,
                                      name=f"pjk1_{half}_{i}", **PJ)
                              for i in range(2)]
            if step < 8:
                ct = step
                for i in range(2):
                    n = half * 2 + i
                    nc.tensor.matmul(
                        pj_ps[key][i][:],
                        lhsT=wk_sb[:, ct, 128:256],
                        rhs=xk_sl[ct][:, n * 512:(n + 1) * 512],
                        start=(ct == 0), stop=(ct == 7),
                    )
            else:
                ps = pj_ps.pop(key)
                for i in range(2):
                    n = half * 2 + i
                    nc.vector.tensor_copy(kTm[1][:, n * 512:(n + 1) * 512],
                                          ps[i][:])

        def out_unit(u):
            st, ec = divmod(u, 2)
            yt = pp.tile([128, 512], f32, name=f"y{st}_{ec}", **PJ)
            for ct in range(2):
                nc.tensor.matmul(
                    yt[:],
                    lhsT=atm[ct][:, st * 128:(st + 1) * 128],
                    rhs=wo_sb[:, ct, ec * 512:(ec + 1) * 512],
                    start=(ct == 0), stop=(ct == 1),
                )
            yo = wk.tile([128, 512], f16, tag="yo", bufs=6,
                         name=f"yo{st}_{ec}")
            nc.vector.tensor_copy(yo[:], yt[:])
            nc.sync.dma_start(
                y[st * 128:(st + 1) * 128, ec * 512:(ec + 1) * 512],
                yo[:])

        # weave schedule: item j -> list of units
        weave_plan = {}
        for st in range(16):                      # v proj: items 10..25
            weave_plan.setdefault(10 + st, []).append(("v", st))
        for u in range(18):                       # k1: items 26..43
            weave_plan.setdefault(26 + u, []).append(("qk1", u))
        # out-proj woven as each qc's norms land (norm chain is ~4us so
        # each window sits ~5 items after that qc's last norm); qc3 trails
        for u in range(8):
            weave_plan.setdefault(88 + 2 * u, []).append(("out", u))
        for u in range(8):
            weave_plan.setdefault(104 + 2 * u, []).append(("out", 8 + u))
        # qc2's atm lands ~slot 120; start 2 items later so these units
        # never head-of-line block the final exps, spilling 2 to the tail
        for u in range(6):
            weave_plan.setdefault(122 + u, []).append(("out", 16 + u))

        def weave(j):
            for kind, u in weave_plan.get(j, ()):
                if kind == "v":
                    v_unit(u)
                elif kind == "qk1":
                    qk1_unit(u)
                else:
                    out_unit(u)

        def norm(h, qc, ot):
            # normalization, off the PE critical path: DVE copy frees the
            # PSUM slot; 128-lane reciprocal via a DRAM-bounce reshape.
            # Whole chain on the gpsimd (SWDGE) queue: the sync queue is
            # congested with y-output DMAs and was adding ~10us of latency
            mt, po = h // 2, 64 * (h % 2)
            # the very last norm (h3, qc3) gates the kernel tail: put its
            # chain on the sync queue so it overlaps h2's gpsimd chain
            eng = nc.sync if (h == 3 and qc == 3) else nc.gpsimd
            ot_sb = wk.tile([65, 512], f32, tag="otsb", bufs=4,
                            name=f"otsb{h}_{qc}")
            nc.vector.tensor_copy(ot_sb[:], ot[:])
            # [1,512] -> [128,4] partition scatter as a single direct
            # SBUF->SBUF DMA (both APs flatten to the same element order)
            d128 = wk.tile([128, 4], f32, tag="d128", bufs=2,
                           name=f"d128_{h}_{qc}")
            eng.dma_start(out=d128[:], in_=ot_sb[64:65, :])
            r128 = wk.tile([128, 4], f32, tag="r128", bufs=2,
                           name=f"r128_{h}_{qc}")
            nc.vector.reciprocal(r128[:], d128[:])
            rd = adr.tile([1, 512], f32, tag="rd", bufs=2,
                          name=f"rd{h}_{qc}")
            eng.dma_start(
                out=rd.rearrange("a (p j) -> (a p) j", j=4), in_=r128[:])
            rec = wk.tile([64, 512], f32, tag="rec", bufs=2,
                          name=f"rec{h}_{qc}")
            eng.dma_start(out=rec[:], in_=rd.broadcast_to([64, 512]))
            stage = wk.tile([64, 512], bf16, tag="stage", bufs=2,
                            name=f"stage{h}_{qc}")
            nc.vector.tensor_mul(stage[:], ot_sb[0:64, :], rec[:])
            eng.dma_start(
                out=atm[mt][po:po + 64, qc * 512:(qc + 1) * 512],
                in_=stage[:])

        # ---------------- unified attention stream ----------------
        items = [(p, qc, kb)
                 for p in range(2) for qc in range(4) for kb in range(16)]
        pts, ots = {}, {}
        n = len(items)

        def emit_pv(i):
            p, qc, kb = items[i]
            pt = pts.pop(i)
            if kb == 0:
                ots[(p, qc)] = [
                    pp.tile([65, 512], f32, name=f"ot{p}_{qc}_{hh}", **OT)
                    for hh in range(2)]
            for hh in range(2):
                nc.tensor.matmul(
                    ots[(p, qc)][hh][:],
                    lhsT=vpst[kb][:, p, hh * 65:(hh + 1) * 65],
                    rhs=pt[:, hh, :],
                    start=(kb == 0), stop=(kb == 15),
                )
            if kb == 15:
                oth = ots.pop((p, qc))
                for hh in range(2):
                    norm(2 * p + hh, qc, oth[hh])

        # PV schedule: large lag early (keeps PVs out of the overloaded
        # v-projection window and protects the exp stream), descending
        # linearly to lag 4 by item 88 so the post-stream tail is short.
        # PVs are emitted BEFORE each item's S matmuls so a stalled S
        # never head-of-line-blocks them.
        def pv_target(j):
            lag = 18 if j < 44 else max(4, 18 - round((j - 43) * 14 / 45))
            return max(0, j - lag + 1)

        pv_ptr = 0
        for j in range(n):
            while pv_ptr < min(pv_target(j), j):
                emit_pv(pv_ptr)
                pv_ptr += 1
            p, qc, kb = items[j]
            weave(j)
            sg = pp.tile([128, 2, 512], f32, name=f"sg{j}", **SG)
            # the two heads run CONCURRENTLY: K=64 row tiles at
            # tile_position (0,0) and (64,0)
            for hh in range(2):
                nc.tensor.matmul(
                    sg[:, hh, :],
                    lhsT=kTm[p][hh * 64:(hh + 1) * 64,
                                kb * 128:(kb + 1) * 128],
                    rhs=qTm[p][hh * 64:(hh + 1) * 64,
                               qc * 512:(qc + 1) * 512],
                    start=True, stop=True,
                )
            pt = wk.tile([128, 2, 512], bf16, tag="pt", bufs=20,
                         name=f"pt{j}")
            nc.scalar.activation(pt[:], sg[:], AF.Exp, scale=SCALE)
            pts[j] = pt
        while pv_ptr < n:
            emit_pv(pv_ptr)
            pv_ptr += 1

        # warm-keeper: discarded matmuls in the freed sg banks keep the
        # PE clock at 8/8 while the final norm chains run (an idle window
        # >3.4us here previously halved the tail matmul rate)
        out_unit(22)
        out_unit(23)
        dmy = pp.tile([128, 2, 512], f32, name="dmy", **SG)
        for i in range(6):
            nc.tensor.matmul(
                dmy[:, i % 2, :],
                lhsT=wq_sb[:, 0, 0:128],
                rhs=xq_sl[0][:, (i % 4) * 512:(i % 4 + 1) * 512],
                start=(i < 2), stop=(i >= 4),
            )
        # trailing output projection (qc 3)
        for u in range(24, 32):
            out_unit(u)

    nc.finalize()
    return nc


def get_nc():
    if "nc" not in _CACHE:
        _CACHE["nc"] = _build_nc()
    return _CACHE["nc"]


def make_in_maps(query, key, value, W_q, W_k, W_v, W_o):
    bf = ml_dtypes.bfloat16

    def t(a):  # contiguous transpose + bf16 cast
        return np.ascontiguousarray(np.asarray(a, np.float32).T).astype(bf)

    xq = {b: t(query[b]) for b in range(B)}
    xk = {b: t(key[b]) for b in range(B)}
    xv = {b: t(value[b]) for b in range(B)}
    W_q, W_k, W_v, W_o = (np.asarray(w, np.float32) for w in (W_q, W_k, W_v, W_o))
    wq = {g: t(W_q[g * DL:(g + 1) * DL, :]) for g in range(4)}
    wk = {g: t(W_k[g * DL:(g + 1) * DL, :]) for g in range(4)}
    wv = {g: t(W_v[g * DL:(g + 1) * DL, :]) for g in range(4)}
    wo = {g: t(W_o[:, g * DL:(g + 1) * DL]) for g in range(4)}

    in_maps = []
    for c in range(NCORES):
        b, g = divmod(c, 4)
        in_maps.append({
            "xqT": xq[b], "xkT": xk[b], "xvT": xv[b],
            "wqT": wq[g], "wkT": wk[g], "wvT": wv[g], "woT": wo[g],
        })
    return in_maps


def combine_outputs(results):
    """results: list of per-core dicts with 'y' -> full (B, S, D) output."""
    outs = [np.asarray(r["y"], np.float32) for r in results]
    return np.stack([
        outs[0] + outs[1] + outs[2] + outs[3],
        outs[4] + outs[5] + outs[6] + outs[7],
    ]).astype(np.float32)


def _exec_cached(nc, in_maps):
    """run_bass_via_pjrt with the jitted executable cached across calls."""
    import jax
    import jax.numpy as jnp  # noqa: F401
    from jax.sharding import Mesh, PartitionSpec
    from jax.experimental.shard_map import shard_map
    import concourse.mybir as mybir
    from concourse import bass2jax

    if "exec" not in _CACHE:
        bass2jax.install_neuronx_cc_hook()
        partition_name = (nc.partition_id_tensor.name
                          if nc.partition_id_tensor else None)
        in_names, out_names, out_avals = [], [], []
        for alloc in nc.m.functions[0].allocations:
            if not isinstance(alloc, mybir.MemoryLocationSet):
                continue
            name = alloc.memorylocations[0].name
            if alloc.kind == "ExternalInput":
                if name != partition_name:
                    in_names.append(name)
            elif alloc.kind == "ExternalOutput":
                out_avals.append(jax.core.ShapedArray(
                    tuple(alloc.tensor_shape), mybir.dt.np(alloc.dtype)))
                out_names.append(name)
        n_params = len(in_names)
        all_names = in_names + out_names
        if partition_name is not None:
            all_names.append(partition_name)
        donate = tuple(range(n_params, n_params + len(out_names)))

        def _body(*args):
            operands = list(args)
            if partition_name is not None:
                operands.append(bass2jax.partition_id_tensor())
            outs = bass2jax._bass_exec_p.bind(
                *operands,
                out_avals=tuple(out_avals),
                in_names=tuple(all_names),
                out_names=tuple(out_names),
                lowering_input_output_aliases=(),
                sim_require_finite=True,
                sim_require_nnan=True,
                nc=nc,
            )
            return tuple(outs)

        mesh = Mesh(np.asarray(jax.devices()[:NCORES]), ("core",))
        specs = (PartitionSpec("core"),) * (n_params + len(out_names))
        out_specs = (PartitionSpec("core"),) * len(out_names)
        _CACHE["exec"] = (
            jax.jit(shard_map(_body, mesh=mesh, in_specs=specs,
                              out_specs=out_specs, check_rep=False),
                    donate_argnums=donate, keep_unused=True),
            in_names, out_names, out_avals,
        )

    sharded, in_names, out_names, out_avals = _CACHE["exec"]
    concat_in = [
        np.concatenate([np.asarray(in_maps[c][name]) for c in range(NCORES)],
                       axis=0)
        for name in in_names
    ]
    concat_zeros = [
        np.zeros((NCORES * a.shape[0], *a.shape[1:]), a.dtype)
        for a in out_avals
    ]
    out_arrs = sharded(*concat_in, *concat_zeros)
    return [
        {name: np.asarray(out_arrs[i]).reshape(
            NCORES, *out_avals[i].shape)[c]
         for i, name in enumerate(out_names)}
        for c in range(NCORES)
    ]


def kernel(query, key, value, W_q, W_k, W_v, W_o):
    nc = get_nc()
    in_maps = make_in_maps(query, key, value, W_q, W_k, W_v, W_o)
    try:
        results = _exec_cached(nc, in_maps)
    except Exception:
        from concourse.bass_utils import run_bass_kernel_spmd
        _CACHE.pop("exec", None)
        results = run_bass_kernel_spmd(nc, in_maps, list(range(NCORES))).results
    return combine_outputs(results)
